# revision 1
# baseline (speedup 1.0000x reference)
"""GCN model (3x GCNConv + LayerNorm + ReLU, mean-pool, 2-layer MLP head)
as a Bass SPMD kernel on 8 Trainium2 NeuronCores — rev2.

Design: nodes (and incident edges, keyed by dst) are partitioned into 8
contiguous blocks. Per layer each core computes y = dinv * (h @ W) for
its block, two AllGathers (A/B sub-table split, ping-pong buffers)
replicate y, then each core aggregates its dst tiles' messages with
dma_gather (bf16 rows, 4 SWDGE queues - the gather path is per-queue
descriptor-rate-bound at ~8.5ns/desc/queue) + selection-matrix matmuls,
LayerNorm + ReLU. Self-loop edges are folded in algebraically from an
SBUF-resident local y block (out = dinv*(gathered + y_local) + b).
Segment mean-pool partial sums are AllReduced before a tiny MLP head.
"""

import numpy as np

import concourse.bass as bass
import concourse.bacc as bacc
import concourse.tile as tile
import concourse.mybir as mybir
from concourse.bass_utils import run_bass_kernel_spmd
from concourse.library_config import mlp as mlp_lib

F32 = mybir.dt.float32
BF16 = mybir.dt.bfloat16
I16 = mybir.dt.int16
I32 = mybir.dt.int32
AF = mybir.ActivationFunctionType
OP = mybir.AluOpType

P = 128


class GCNConfig:
    def __init__(self, N=50000, E=800000, F_IN=128, H=256, G=64, A=8, OUT=1,
                 M=8, TA=25):
        self.N, self.E, self.F_IN, self.H, self.G, self.A, self.OUT, self.M = \
            N, E, F_IN, H, G, A, OUT, M
        self.NL = N // M                      # nodes per core
        assert self.NL * M == N
        self.T = (self.NL + P - 1) // P       # dst tiles per core
        self.NP = self.T * P                  # padded nodes per core
        self.TA = TA                          # tiles in sub-table A
        self.TB = self.T - TA
        self.RA_L = TA * P                    # A rows per core
        self.RB_L = self.TB * P
        self.RA = self.RA_L * M               # A sub-table rows (all cores)
        self.RB = self.RB_L * M
        assert self.RA < 32768 and self.RB < 32768
        self.KH = H // P


CFG = GCNConfig()


def _wrap_idx16(vals):
    """[n] -> [128, n//16] in the 16-partition-wrapped, 8x-replicated
    layout dma_gather expects (element i at [i % 16, i // 16])."""
    n = vals.shape[0]
    assert n % 16 == 0
    arr = vals.reshape(n // 16, 16).T.astype(np.int16)
    return np.tile(arr, (8, 1))


def preprocess(cfg, x, edge_index, batch):
    """Host-side sharding. Real edges only (no self-loops); sources mapped
    into the A/B sub-table layout; per-core exact counts for runtime
    num_idxs."""
    N, E, M, NL, NP, T = cfg.N, cfg.E, cfg.M, cfg.NL, cfg.NP, cfg.T
    TA, RA_L, RB_L = cfg.TA, cfg.RA_L, cfg.RB_L
    src = np.asarray(edge_index[0], dtype=np.int64)
    dst = np.asarray(edge_index[1], dtype=np.int64)
    batch = np.asarray(batch, dtype=np.int64)

    deg = np.bincount(dst, minlength=N).astype(np.float64) + 1.0
    dinv = (1.0 / np.sqrt(deg)).astype(np.float32)

    core = dst // NL
    dloc = dst % NL
    tile_id = dloc // P
    dslot_v = (dloc % P).astype(np.float32)
    sc = src // NL
    sl = src % NL
    sub = (sl >= RA_L).astype(np.int64)                  # 0 = A, 1 = B
    row = np.where(sub == 0, sc * RA_L + sl, sc * RB_L + (sl - RA_L))

    # sort into (core, tile, sub) blocks
    key = (core * T + tile_id) * 2 + sub
    order = np.argsort(key, kind="stable")
    key_s = key[order]
    row_s = row[order]
    dslot_s = dslot_v[order]

    nkeys = M * T * 2
    counts = np.bincount(key_s, minlength=nkeys).reshape(M, T, 2)
    starts = np.zeros(nkeys + 1, dtype=np.int64)
    np.cumsum(counts.reshape(-1), out=starts[1:])

    J = np.ceil(counts.max(axis=0) / P).astype(np.int64)     # [T, 2] caps
    JMAX = int(J.sum(axis=1).max())

    W16 = int(J.sum() * 8)
    WD = int(J.sum())
    per_core = []
    for c in range(M):
        idx16 = np.zeros((P, W16), np.int16)
        dslot = np.full((P, WD), 300.0, np.float32)
        cnts = np.zeros(2 * T, np.int32)
        o16 = 0
        od = 0
        for t in range(T):
            for h in range(2):
                Jth = int(J[t, h])
                if Jth == 0:
                    continue
                Cap = Jth * P
                k = (c * T + t) * 2 + h
                s, e = starts[k], starts[k + 1]
                n = e - s
                vi = np.zeros(Cap, np.int64)
                vd = np.full(Cap, 300.0, np.float32)
                vi[:n] = row_s[s:e]
                vd[:n] = dslot_s[s:e]
                idx16[:, o16:o16 + Jth * 8] = _wrap_idx16(vi)
                dslot[:, od:od + Jth] = vd.reshape(Jth, P).T
                cnts[2 * t + h] = n
                o16 += Jth * 8
                od += Jth
        dpad = np.zeros(NP, np.float32)
        dpad[:NL] = dinv[c * NL:(c + 1) * NL]
        dinvT = dpad.reshape(T, P).T.copy()              # [128, T]
        pm = np.zeros((NP, cfg.G), np.float32)
        pm[np.arange(NL), batch[c * NL:(c + 1) * NL]] = 1.0
        poolm = pm.reshape(T, P, cfg.G).transpose(1, 0, 2).copy()
        xT = np.zeros((cfg.F_IN, NP), np.float32)
        xT[:, :NL] = np.asarray(x[c * NL:(c + 1) * NL], np.float32).T
        per_core.append(dict(idx16=idx16, dslot=dslot, dinvT=dinvT,
                             cnts=cnts.reshape(1, 2 * T),
                             poolm=poolm.reshape(P, T * cfg.G), xT=xT))

    meta = dict(J=J, JMAX=JMAX, W16=W16, WD=WD)
    return per_core, meta


def build_program(cfg, meta, gamma_trivial, beta_trivial, ablate=(), nbuf=4,
                  nq=4):
    ablate = frozenset(ablate)
    N, M, T, NP, H, G, A = cfg.N, cfg.M, cfg.T, cfg.NP, cfg.H, cfg.G, cfg.A
    TA, RA_L, RB_L, RA, RB = cfg.TA, cfg.RA_L, cfg.RB_L, cfg.RA, cfg.RB
    J = meta["J"]
    JMAX = meta["JMAX"]
    KH = cfg.KH

    nc = bacc.Bacc("TRN2", target_bir_lowering=False, debug=False,
                   num_devices=M, num_swdge_queues=nq)

    def din(name, shape, dt=F32):
        return nc.dram_tensor(name, shape, dt, kind="ExternalInput").ap()

    xT_ap = din("xT", [cfg.F_IN, NP])
    idx16_ap = din("idx16", [P, meta["W16"]], I16)
    dslot_ap = din("dslot", [P, meta["WD"]], BF16)
    cnts_ap = din("cnts", [1, 2 * T], I32)
    dinvT_ap = din("dinvT", [P, T])
    poolm_ap = din("poolm", [P, T * G])
    bgb_ap = din("bgb", [P, 9, H])
    iota_ap = din("iota_in", [P, JMAX * P], BF16)
    ident_ap = din("ident_in", [P, P])
    W1_ap = din("W1", [cfg.F_IN, H])
    W2_ap = din("W2", [H, H], BF16)
    W3_ap = din("W3", [H, H], BF16)
    fc1_ap = din("fc1aug", [3 * P, H])
    attr_ap = din("attraug", [P, G])
    invc_ap = din("invc", [G, 1])
    fcw2_ap = din("fcw2row", [1, H])
    fcb2_ap = din("fcb2col", [G, 1])
    out_ap = nc.dram_tensor("out", [G, cfg.OUT], F32,
                            kind="ExternalOutput").ap()

    y_cc_a = nc.dram_tensor("y_cc_a", [RA_L, H], BF16)
    y_cc_b = nc.dram_tensor("y_cc_b", [RB_L, H], BF16)
    # ping-pong by layer parity so the next layer's AllGather (issued
    # mid-aggregation) never clobbers the table the current layer's
    # gathers still read — otherwise Tile serializes AG behind them
    y_full_a = [nc.dram_tensor(f"y_full_a{i}", [RA, H], BF16,
                               addr_space="Shared") for i in range(2)]
    y_full_b = [nc.dram_tensor(f"y_full_b{i}", [RB, H], BF16,
                               addr_space="Shared") for i in range(2)]
    pool_in = nc.dram_tensor("pool_in", [G, H], F32)
    pool_out = nc.dram_tensor("pool_out", [G, H], F32, addr_space="Shared")

    rg = [list(range(M))]

    na = 3 if "psum3" in ablate else 2
    npt = 1 if "psum3" in ablate else 2
    with tile.TileContext(nc) as tc:
        with tc.tile_pool(name="const", bufs=1) as cst, \
             tc.tile_pool(name="sbw", bufs=4) as sbw, \
             tc.tile_pool(name="msgp", bufs=nbuf) as msgp, \
             tc.tile_pool(name="sp", bufs=nbuf) as sp, \
             tc.tile_pool(name="small", bufs=8) as small, \
             tc.tile_pool(name="psp_a", bufs=na, space="PSUM") as psp_a, \
             tc.tile_pool(name="psp_y", bufs=2, space="PSUM") as psp_y, \
             tc.tile_pool(name="pst", bufs=npt, space="PSUM") as pst, \
             tc.tile_pool(name="ptail", bufs=2, space="PSUM") as ptail:

            nc.gpsimd.load_library(mlp_lib)

            # ---- constants
            idx16_sb = cst.tile([P, meta["W16"]], I16)
            nc.sync.dma_start(out=idx16_sb[:], in_=idx16_ap[:])
            dslot_sb = cst.tile([P, meta["WD"]], BF16)
            nc.sync.dma_start(out=dslot_sb[:], in_=dslot_ap[:])
            cnts_sb = cst.tile([1, 2 * T], I32)
            nc.sync.dma_start(out=cnts_sb[:], in_=cnts_ap[:])
            dinv_sb = cst.tile([P, T], F32)
            nc.sync.dma_start(out=dinv_sb[:], in_=dinvT_ap[:])
            poolm_sb = cst.tile([P, T, G], F32)
            nc.sync.dma_start(out=poolm_sb[:],
                              in_=poolm_ap[:].rearrange("p (t g) -> p t g", g=G))
            bgb_sb = cst.tile([P, 9, H], F32)
            nc.sync.dma_start(out=bgb_sb[:], in_=bgb_ap[:])
            iota_sb = cst.tile([P, JMAX * P], BF16)
            nc.sync.dma_start(out=iota_sb[:], in_=iota_ap[:])
            ident_sb = cst.tile([P, P], F32)
            nc.sync.dma_start(out=ident_sb[:], in_=ident_ap[:])
            W1_sb = cst.tile([cfg.F_IN, H], F32)
            nc.sync.dma_start(out=W1_sb[:], in_=W1_ap[:])
            W2_sb = cst.tile([P, KH, H], BF16)
            nc.sync.dma_start(out=W2_sb[:],
                              in_=W2_ap[:].rearrange("(k p) h -> p k h", p=P))
            W3_sb = cst.tile([P, KH, H], BF16)
            nc.sync.dma_start(out=W3_sb[:],
                              in_=W3_ap[:].rearrange("(k p) h -> p k h", p=P))
            fc1_sb = cst.tile([P, 3, H], F32)
            nc.sync.dma_start(out=fc1_sb[:],
                              in_=fc1_ap[:].rearrange("(k p) h -> p k h", p=P))
            attr_sb = cst.tile([P, G], F32)
            nc.sync.dma_start(out=attr_sb[:], in_=attr_ap[:])
            invc_sb = cst.tile([G, 1], F32)
            nc.sync.dma_start(out=invc_sb[:], in_=invc_ap[:])
            fcw2_sb = cst.tile([1, H], F32)
            nc.sync.dma_start(out=fcw2_sb[:], in_=fcw2_ap[:])
            fcb2_sb = cst.tile([G, 1], F32)
            nc.sync.dma_start(out=fcb2_sb[:], in_=fcb2_ap[:])

            eps_sb = cst.tile([P, 1], F32)
            nc.vector.memset(eps_sb[:], 1e-5)
            ones_sb = cst.tile([1, G], F32)
            nc.vector.memset(ones_sb[:], 1.0)
            pool_acc = cst.tile([G, H], F32)
            nc.vector.memset(pool_acc[:], 0.0)
            ylocal = cst.tile([P, T, H], BF16)       # this core's y rows
            hT_sb = cst.tile([P, T * KH, P], BF16)

            # zero-init all msg buffers: capacity gathers always rewrite
            # them, but JMAX-Jt tail chunks of shorter tiles stay stale and
            # NaN * 0 = NaN on uninitialized SBUF
            for _ in range(nbuf):
                m0 = msgp.tile([P, JMAX, H], BF16, tag="msg")
                nc.vector.memset(m0[:], 0.0)

            regs = {}
            for v in sorted({int(J[t, h]) * P for t in range(T)
                             for h in range(2) if J[t, h] > 0} | {P}):
                regs[v] = nc.gpsimd.to_reg(v)

            # block offsets into idx16/dslot slabs
            o16 = np.zeros((T, 2), np.int64)
            od = np.zeros((T, 2), np.int64)
            acc16 = 0
            accd = 0
            for t in range(T):
                for h in range(2):
                    o16[t, h] = acc16
                    od[t, h] = accd
                    acc16 += int(J[t, h]) * 8
                    accd += int(J[t, h])

            Wsb = [W1_sb, W2_sb, W3_sb]
            gq = [0]

            def emit_phase_a(L, t):
                # ylocal[:, t] = dinv * (h_{L-1} @ W_L); layer 0 reads xT
                psy = psp_y.tile([P, H], F32, tag="psy")
                if L == 0:
                    xt = sbw.tile([P, P], F32, tag="xt")
                    nc.sync.dma_start(out=xt[:],
                                      in_=xT_ap[:, t * P:(t + 1) * P])
                    nc.tensor.matmul(psy[:], lhsT=xt[:], rhs=W1_sb[:],
                                     start=True, stop=True)
                else:
                    for kk in range(KH):
                        nc.tensor.matmul(
                            psy[:], lhsT=hT_sb[:, t * KH + kk, :],
                            rhs=Wsb[L][:, kk, :],
                            start=(kk == 0), stop=(kk == KH - 1))
                nc.scalar.mul(out=ylocal[:, t, :], in_=psy[:],
                              mul=dinv_sb[:, t:t + 1])
                if t < TA:
                    nc.sync.dma_start(out=y_cc_a[t * P:(t + 1) * P, :],
                                      in_=ylocal[:, t, :])
                else:
                    tb = t - TA
                    nc.sync.dma_start(out=y_cc_b[tb * P:(tb + 1) * P, :],
                                      in_=ylocal[:, t, :])

            def emit_ag(which, par):
                if "noag" in ablate:
                    return
                if which == 0:
                    nc.gpsimd.collective_compute(
                        "AllGather", OP.bypass, replica_groups=rg,
                        ins=[y_cc_a[:]], outs=[y_full_a[par][:]])
                else:
                    nc.gpsimd.collective_compute(
                        "AllGather", OP.bypass, replica_groups=rg,
                        ins=[y_cc_b[:]], outs=[y_full_b[par][:]])

            for t in range(T):
                emit_phase_a(0, t)
                if t == TA - 1:
                    emit_ag(0, 0)
            emit_ag(1, 0)

            for L in range(3):
                for t in range(T):
                    Ja, Jb = int(J[t, 0]), int(J[t, 1])
                    Jt = Ja + Jb
                    msg = msgp.tile([P, JMAX, H], BF16, tag="msg")
                    Jag = min(Ja, 1) if "smallgather" in ablate else Ja
                    Jbg = min(Jb, 1) if "smallgather" in ablate else Jb
                    if Ja > 0:
                        nc.gpsimd.dma_gather(
                            out_ap=msg[:, :Jag, :], in_ap=y_full_a[L % 2][:],
                            idxs_ap=idx16_sb[:, int(o16[t, 0]):int(o16[t, 0]) + Jag * 8],
                            num_idxs=Jag * P, num_idxs_reg=regs[Jag * P],
                            elem_size=H, single_packet=False,
                            queue_num=gq[0] % nq)
                        gq[0] += 1
                    if Jb > 0:
                        nc.gpsimd.dma_gather(
                            out_ap=msg[:, Ja:Ja + Jbg, :], in_ap=y_full_b[L % 2][:],
                            idxs_ap=idx16_sb[:, int(o16[t, 1]):int(o16[t, 1]) + Jbg * 8],
                            num_idxs=Jbg * P, num_idxs_reg=regs[Jbg * P],
                            elem_size=H, single_packet=False,
                            queue_num=gq[0] % nq)
                        gq[0] += 1
                    ps = psp_a.tile([P, H], F32, tag="agg")
                    a = int(od[t, 0])
                    S_all = sp.tile([P, JMAX, P], BF16, tag="S")
                    dsl = dslot_sb[:, a:a + Jt]
                    dsl_b = bass.AP(tensor=dsl.tensor, offset=dsl.offset,
                                    ap=[dsl.ap[0], dsl.ap[1], [0, P]])
                    nc.vector.tensor_tensor(
                        out=S_all[:, :Jt, :],
                        in0=iota_sb[:, :Jt * P].rearrange(
                            "p (j d) -> p j d", d=P),
                        in1=dsl_b,
                        op=OP.is_equal)
                    Jn = 1 if "noagg" in ablate else Jt
                    for j in range(Jn):
                        nc.tensor.matmul(ps[:], lhsT=S_all[:, j, :],
                                         rhs=msg[:, j, :],
                                         start=(j == 0), stop=(j == Jn - 1))

                    # ---------- evict + self-loop + bias + LN + relu ----
                    tt = sbw.tile([P, H], F32, tag="tt")
                    nc.vector.tensor_add(out=tt[:], in0=ps[:],
                                         in1=ylocal[:, t, :])
                    nc.vector.tensor_scalar_mul(out=tt[:], in0=tt[:],
                                                scalar1=dinv_sb[:, t:t + 1])
                    nc.vector.tensor_add(out=tt[:], in0=tt[:],
                                         in1=bgb_sb[:, 3 * L + 0, :])
                    if "noln" not in ablate:
                        stats = small.tile([P, 6], F32, tag="stats")
                        nc.vector.bn_stats(out=stats[:], in_=tt[:])
                        mv = small.tile([P, 2], F32, tag="mv")
                        nc.vector.bn_aggr(out=mv[:], in_=stats[:])
                        rstd = small.tile([P, 1], F32, tag="rstd")
                        nc.scalar.activation(out=rstd[:], in_=mv[:, 1:2],
                                             func=AF.Sqrt, bias=eps_sb[:],
                                             scale=1.0)
                        nc.vector.reciprocal(out=rstd[:], in_=rstd[:])
                        nc.vector.tensor_scalar(
                            out=tt[:], in0=tt[:], scalar1=mv[:, 0:1],
                            scalar2=rstd[:], op0=OP.subtract, op1=OP.mult)
                    if not gamma_trivial:
                        nc.vector.tensor_mul(out=tt[:], in0=tt[:],
                                             in1=bgb_sb[:, 3 * L + 1, :])
                    if not beta_trivial:
                        nc.vector.tensor_add(out=tt[:], in0=tt[:],
                                             in1=bgb_sb[:, 3 * L + 2, :])
                    h_t = sbw.tile([P, H], F32, tag="h")
                    nc.scalar.activation(out=h_t[:], in_=tt[:], func=AF.Relu)

                    if L < 2:
                        for kk in range(KH):
                            pt = pst.tile([P, P], F32, tag="pt")
                            nc.tensor.transpose(
                                out=pt[:], in_=h_t[:, kk * P:(kk + 1) * P],
                                identity=ident_sb[:])
                            nc.vector.tensor_copy(
                                out=hT_sb[:, t * KH + kk, :], in_=pt[:])
                        emit_phase_a(L + 1, t)
                        if t == TA - 1:
                            emit_ag(0, (L + 1) % 2)
                        elif t == T - 1:
                            emit_ag(1, (L + 1) % 2)
                    else:
                        pp = ptail.tile([G, H], F32, tag="tail")
                        nc.tensor.matmul(pp[:], lhsT=poolm_sb[:, t, :],
                                         rhs=h_t[:], start=True, stop=True)
                        nc.vector.tensor_add(out=pool_acc[:], in0=pool_acc[:],
                                             in1=pp[:])

            # ---------- pooled mean + MLP head ----------
            nc.sync.dma_start(out=pool_in[:], in_=pool_acc[:])
            nc.gpsimd.collective_compute(
                "AllReduce", OP.add, replica_groups=rg,
                ins=[pool_in[:]], outs=[pool_out[:]])
            pooled = sbw.tile([G, H], F32, tag="pooled")
            nc.sync.dma_start(out=pooled[:], in_=pool_out[:])
            nc.vector.tensor_scalar_mul(out=pooled[:], in0=pooled[:],
                                        scalar1=invc_sb[:])
            zt = sbw.tile([P, KH, G], F32, tag="zt")
            for kk in range(KH):
                pz = ptail.tile([P, G], F32, tag="tail")
                nc.tensor.transpose(out=pz[:], in_=pooled[:, kk * P:(kk + 1) * P],
                                    identity=ident_sb[:G, :G])
                nc.vector.tensor_copy(out=zt[:, kk, :], in_=pz[:])
            ups = ptail.tile([G, H], F32, tag="tail")
            nc.tensor.matmul(ups[:], lhsT=zt[:, 0, :], rhs=fc1_sb[:, 0, :],
                             start=True, stop=False)
            nc.tensor.matmul(ups[:], lhsT=zt[:, 1, :], rhs=fc1_sb[:, 1, :],
                             start=False, stop=False)
            nc.tensor.matmul(ups[:], lhsT=attr_sb[:], rhs=fc1_sb[:, 2, :],
                             start=False, stop=True)
            r = sbw.tile([G, H], F32, tag="r")
            nc.scalar.activation(out=r[:], in_=ups[:], func=AF.Relu)
            wps = ptail.tile([G, H], F32, tag="tail")
            nc.tensor.matmul(wps[:], lhsT=ones_sb[:], rhs=fcw2_sb[:],
                             start=True, stop=True)
            rr = sbw.tile([G, H], F32, tag="rr")
            nc.vector.tensor_mul(out=rr[:], in0=r[:], in1=wps[:])
            o = small.tile([G, 1], F32, tag="o")
            nc.vector.tensor_reduce(out=o[:], in_=rr[:],
                                    axis=mybir.AxisListType.X, op=OP.add)
            nc.vector.tensor_scalar_add(out=o[:], in0=o[:],
                                        scalar1=fcb2_sb[:])
            nc.sync.dma_start(out=out_ap[:], in_=o[:])

    nc.compile()
    return nc


def make_in_maps(cfg, inputs, per_core, meta):
    import ml_dtypes
    H, G, A = cfg.H, cfg.G, cfg.A
    f = lambda a: np.ascontiguousarray(np.asarray(a, np.float32))
    W1, b1 = f(inputs["W1"]), f(inputs["b1"])
    W2, b2 = f(inputs["W2"]), f(inputs["b2"])
    W3, b3 = f(inputs["W3"]), f(inputs["b3"])
    g1, be1 = f(inputs["g1"]), f(inputs["be1"])
    g2, be2 = f(inputs["g2"]), f(inputs["be2"])
    g3, be3 = f(inputs["g3"]), f(inputs["be3"])
    fcW1, fcb1 = f(inputs["fcW1"]), f(inputs["fcb1"])
    fcW2, fcb2 = f(inputs["fcW2"]), f(inputs["fcb2"])
    graph_attr = f(inputs["graph_attr"]).reshape(-1, A)
    batch = np.asarray(inputs["batch"], np.int64)

    bgb = np.zeros((P, 9, H), np.float32)
    for i, v in enumerate([b1, g1, be1, b2, g2, be2, b3, g3, be3]):
        bgb[:, i, :] = v[None, :]
    fc1aug = np.zeros((3 * P, H), np.float32)
    fc1aug[:H, :] = fcW1[:H, :]
    fc1aug[2 * P:2 * P + A, :] = fcW1[H:H + A, :]
    fc1aug[2 * P + A, :] = fcb1
    attraug = np.zeros((P, G), np.float32)
    attraug[:A, :] = graph_attr.T
    attraug[A, :] = 1.0
    cnt = np.bincount(batch, minlength=G).astype(np.float32)
    invc = (1.0 / np.maximum(cnt, 1.0)).reshape(G, 1).astype(np.float32)
    fcw2row = fcW2[:, 0].reshape(1, H).copy()
    fcb2col = np.full((G, 1), fcb2[0], np.float32)
    iota_in = np.tile(np.arange(P, dtype=np.float32),
                      (P, meta["JMAX"])).astype(ml_dtypes.bfloat16)
    ident_in = np.eye(P, dtype=np.float32)

    shared = dict(bgb=bgb, iota_in=iota_in, ident_in=ident_in, W1=W1,
                  W2=W2.astype(ml_dtypes.bfloat16),
                  W3=W3.astype(ml_dtypes.bfloat16),
                  fc1aug=fc1aug, attraug=attraug, invc=invc,
                  fcw2row=fcw2row, fcb2col=fcb2col)
    in_maps = []
    for c in range(cfg.M):
        m = dict(shared)
        m.update(per_core[c])
        m["dslot"] = m["dslot"].astype(ml_dtypes.bfloat16)
        in_maps.append(m)
    return in_maps


_CACHE = {}


def _get_program(cfg, meta, gamma_trivial, beta_trivial):
    key = (tuple(meta["J"].reshape(-1).tolist()), gamma_trivial, beta_trivial)
    if key not in _CACHE:
        _CACHE[key] = build_program(cfg, meta, gamma_trivial, beta_trivial)
    return _CACHE[key]


def run(cfg, inputs, nc=None):
    per_core, meta = preprocess(cfg, inputs["x"], inputs["edge_index"],
                                inputs["batch"])
    gamma_trivial = all(np.allclose(np.asarray(inputs[k]), 1.0)
                        for k in ("g1", "g2", "g3"))
    beta_trivial = all(np.allclose(np.asarray(inputs[k]), 0.0)
                       for k in ("be1", "be2", "be3"))
    if nc is None:
        nc = _get_program(cfg, meta, gamma_trivial, beta_trivial)
    in_maps = make_in_maps(cfg, inputs, per_core, meta)
    res = None
    for attempt in range(3):
        try:
            res = run_bass_kernel_spmd(nc, in_maps, list(range(cfg.M)))
            break
        except Exception:
            if attempt == 2:
                raise
    return res.results[0]["out"].astype(np.float32)


def kernel(**inputs) -> np.ndarray:
    return run(CFG, inputs)



# revision 2
# speedup vs baseline: 1.3425x; 1.3425x over previous
"""GCN model (3x GCNConv + LayerNorm + ReLU, mean-pool, 2-layer MLP head)
as a Bass SPMD kernel on 8 Trainium2 NeuronCores — rev3.

Aggregate-then-transform design: each layer gathers dinv-prescaled table
rows (x in bf16 for layer 0 — staged per-core, no AllGather; h in f8e3 for
layers 1-2, AllGathered in two sub-table chunks), aggregates them with
host-precomputed one-hot S matrices in swapped-operand matmuls that
produce the transposed sum zT directly (no PE transposes), then applies
W via 2 matmuls + a 1-partition bias matmul (b scaled by 1/dinv so the
later per-partition dinv scale is exact), LayerNorm + ReLU fused into one
ACT op with per-partition scale/bias. Self-loops read the local scaled-h
block directly as an extra identity-S matmul chunk (no descriptors).
A-gathers run a few tiles ahead of B-gathers to cover AG-B exposure at
layer boundaries.
"""

import numpy as np

import concourse.bass as bass
import concourse.bacc as bacc
import concourse.tile as tile
import concourse.mybir as mybir
from concourse.bass_utils import run_bass_kernel_spmd
from concourse.library_config import mlp as mlp_lib

F32 = mybir.dt.float32
BF16 = mybir.dt.bfloat16
F8 = mybir.dt.float8e3
I16 = mybir.dt.int16
I32 = mybir.dt.int32
AF = mybir.ActivationFunctionType
OP = mybir.AluOpType

P = 128


class GCNConfig:
    def __init__(self, N=50000, E=800000, F_IN=128, H=256, G=64, A=8, OUT=1,
                 M=8, TA=31, LA=6):
        self.N, self.E, self.F_IN, self.H, self.G, self.A, self.OUT, self.M = \
            N, E, F_IN, H, G, A, OUT, M
        self.NL = N // M                      # nodes per core
        assert self.NL * M == N
        self.T = (self.NL + P - 1) // P       # dst tiles per core
        self.NP = self.T * P                  # padded nodes per core
        self.TA = TA                          # tiles in sub-table A
        self.TB = self.T - TA
        self.RA_L = TA * P                    # A rows per core
        self.RB_L = self.TB * P
        self.RA = self.RA_L * M               # A sub-table rows (all cores)
        self.RB = self.RB_L * M
        assert self.RA < 32768 and self.RB < 32768
        self.LA = LA                          # A-gather lookahead (tiles)


CFG = GCNConfig()


def _wrap_idx16(vals):
    """[n] -> [128, n//16] in the 16-partition-wrapped, 8x-replicated
    layout dma_gather expects (element i at [i % 16, i // 16])."""
    n = vals.shape[0]
    assert n % 16 == 0
    arr = vals.reshape(n // 16, 16).T.astype(np.int16)
    return np.tile(arr, (8, 1))


def preprocess(cfg, x, edge_index, batch):
    """Host-side sharding: per-core idx16 slabs, one-hot S slabs (bf16 for
    layer 0, f8e3 for layers 1-2), prescaled x gather tables, local x
    blocks, dinv columns, pool matrices."""
    import ml_dtypes
    N, E, M, NL, NP, T = cfg.N, cfg.E, cfg.M, cfg.NL, cfg.NP, cfg.T
    TA, RA_L, RB_L = cfg.TA, cfg.RA_L, cfg.RB_L
    G = cfg.G
    src = np.asarray(edge_index[0], dtype=np.int64)
    dst = np.asarray(edge_index[1], dtype=np.int64)
    batch = np.asarray(batch, dtype=np.int64)

    deg = np.bincount(dst, minlength=N).astype(np.float64) + 1.0
    dinv = (1.0 / np.sqrt(deg)).astype(np.float32)

    core = dst // NL
    dloc = dst % NL
    tile_id = dloc // P
    dslot_v = (dloc % P).astype(np.int64)
    sc = src // NL
    sl = src % NL
    sub = (sl >= RA_L).astype(np.int64)                  # 0 = A, 1 = B
    row = np.where(sub == 0, sc * RA_L + sl, sc * RB_L + (sl - RA_L))

    key = (core * T + tile_id) * 2 + sub
    order = np.argsort(key, kind="stable")
    key_s = key[order]
    row_s = row[order]
    dslot_s = dslot_v[order]

    nkeys = M * T * 2
    counts = np.bincount(key_s, minlength=nkeys).reshape(M, T, 2)
    starts = np.zeros(nkeys + 1, dtype=np.int64)
    np.cumsum(counts.reshape(-1), out=starts[1:])

    J = np.ceil(counts.max(axis=0) / P).astype(np.int64)     # [T, 2] caps
    NCH = int((J.sum(axis=1) + 1).sum())                     # S chunks total
    JTMAX = int((J.sum(axis=1) + 1).max())

    W16 = int(J.sum() * 8)
    # prescaled x gather tables (identical on every core)
    xs = (np.asarray(x, np.float32) * dinv[:, None])
    xa = np.zeros((cfg.RA, cfg.F_IN), np.float32)
    xb = np.zeros((cfg.RB, cfg.F_IN), np.float32)
    for c in range(M):
        loc = xs[c * NL:(c + 1) * NL]
        xa[c * RA_L:(c + 1) * RA_L] = loc[:RA_L]
        xb[c * RB_L:c * RB_L + (NL - RA_L)] = loc[RA_L:]
    xa = xa.astype(ml_dtypes.bfloat16)
    xb = xb.astype(ml_dtypes.bfloat16)

    per_core = []
    for c in range(M):
        idx16 = np.zeros((P, W16), np.int16)
        Sslab = np.zeros((P, NCH, P), np.float32)
        o16 = 0
        och = 0
        for t in range(T):
            for h in range(2):
                Jth = int(J[t, h])
                if Jth == 0:
                    continue
                Cap = Jth * P
                k = (c * T + t) * 2 + h
                s, e = starts[k], starts[k + 1]
                n = e - s
                vi = np.zeros(Cap, np.int64)
                vi[:n] = row_s[s:e]
                idx16[:, o16:o16 + Jth * 8] = _wrap_idx16(vi)
                q = np.arange(n)
                Sslab[q % P, och + q // P, dslot_s[s:e]] = 1.0
                o16 += Jth * 8
                och += Jth
            # self chunk: identity
            Sslab[np.arange(P), och, np.arange(P)] = 1.0
            och += 1
        assert och == NCH
        x0loc = np.zeros((P, T, cfg.F_IN), np.float32)
        x0loc.reshape(NP, cfg.F_IN)[:NL] = xs[c * NL:(c + 1) * NL]
        dpad = np.zeros(NP, np.float32)
        dpad[:NL] = dinv[c * NL:(c + 1) * NL]
        dinvT = dpad.reshape(T, P).T.copy()              # [128, T]
        dinv2T = (dinvT * dinvT).copy()
        ipad = np.zeros(NP, np.float32)
        ipad[:NL] = 1.0 / dinv[c * NL:(c + 1) * NL]      # sqrt(deg)
        invd = ipad.reshape(1, NP)
        pm = np.zeros((NP, G), np.float32)
        pm[np.arange(NL), batch[c * NL:(c + 1) * NL]] = 1.0
        poolm = pm.reshape(T, P, G).transpose(1, 0, 2).copy()
        per_core.append(dict(
            idx16=idx16,
            S0=Sslab.reshape(P, NCH * P).astype(ml_dtypes.bfloat16),
            S12=Sslab.reshape(P, NCH * P).astype(ml_dtypes.float8_e3m4),
            x0loc=x0loc.reshape(P, T * cfg.F_IN).astype(ml_dtypes.bfloat16),
            dinvT=dinvT, dinv2T=dinv2T,
            invd=invd.astype(ml_dtypes.bfloat16),
            poolm=poolm.reshape(P, T * G).astype(ml_dtypes.bfloat16),
            xa=xa, xb=xb))

    meta = dict(J=J, NCH=NCH, JTMAX=JTMAX, W16=W16)
    return per_core, meta


def build_program(cfg, meta, gamma_trivial, beta_trivial, ablate=(), nq=4):
    ablate = frozenset(ablate)
    N, M, T, NP, H, G, A = cfg.N, cfg.M, cfg.T, cfg.NP, cfg.H, cfg.G, cfg.A
    TA, RA_L, RB_L, RA, RB = cfg.TA, cfg.RA_L, cfg.RB_L, cfg.RA, cfg.RB
    F0, LA = cfg.F_IN, cfg.LA
    J = meta["J"]
    NCH = meta["NCH"]
    JTMAX = meta["JTMAX"]

    nc = bacc.Bacc("TRN2", target_bir_lowering=False, debug=False,
                   num_devices=M, num_swdge_queues=nq)

    def din(name, shape, dt=F32):
        return nc.dram_tensor(name, shape, dt, kind="ExternalInput").ap()

    idx16_ap = din("idx16", [P, meta["W16"]], I16)
    S0_ap = din("S0", [P, NCH * P], BF16)
    S12_ap = din("S12", [P, NCH * P], F8)
    xa_ap = din("xa", [RA, F0], BF16)
    xb_ap = din("xb", [RB, F0], BF16)
    x0loc_ap = din("x0loc", [P, T * F0], BF16)
    dinvT_ap = din("dinvT", [P, T])
    dinv2T_ap = din("dinv2T", [P, T])
    invd_ap = din("invd", [1, NP], BF16)
    brows_ap = din("brows", [1, 3 * H], BF16)
    gb_ap = din("gb", [P, 6, H])
    W1_ap = din("W1", [P, H], BF16)
    W2_ap = din("W2", [H, H], BF16)
    W3_ap = din("W3", [H, H], BF16)
    poolm_ap = din("poolm", [P, T * G], BF16)
    fc1_ap = din("fc1aug", [3 * P, H])
    attr_ap = din("attraug", [P, G])
    invc_ap = din("invc", [G, 1])
    fcw2_ap = din("fcw2row", [1, H])
    fcb2_ap = din("fcb2col", [G, 1])
    ident_ap = din("ident_in", [P, P])
    out_ap = nc.dram_tensor("out", [G, cfg.OUT], F32,
                            kind="ExternalOutput").ap()

    h_cc_a = [nc.dram_tensor(f"h{L}_cc_a", [RA_L, H], F8) for L in range(2)]
    h_cc_b = [nc.dram_tensor(f"h{L}_cc_b", [RB_L, H], F8) for L in range(2)]
    h_full_a = [nc.dram_tensor(f"h{L}_full_a", [RA, H], F8,
                               addr_space="Shared") for L in range(2)]
    h_full_b = [nc.dram_tensor(f"h{L}_full_b", [RB, H], F8,
                               addr_space="Shared") for L in range(2)]
    pool_in = nc.dram_tensor("pool_in", [G, H], F32)
    pool_out = nc.dram_tensor("pool_out", [G, H], F32, addr_space="Shared")

    rg = [list(range(M))]
    KH = H // P

    with tile.TileContext(nc) as tc:
        with tc.tile_pool(name="const", bufs=1) as cst, \
             tc.tile_pool(name="sbw", bufs=4) as sbw, \
             tc.tile_pool(name="msgA", bufs=LA + 3) as msgA_p, \
             tc.tile_pool(name="msgB", bufs=4) as msgB_p, \
             tc.tile_pool(name="sp", bufs=4) as sp, \
             tc.tile_pool(name="small", bufs=12) as small, \
             tc.tile_pool(name="psz", bufs=2, space="PSUM") as psz, \
             tc.tile_pool(name="psy", bufs=2, space="PSUM") as psyp, \
             tc.tile_pool(name="ptail", bufs=2, space="PSUM") as ptail:

            nc.gpsimd.load_library(mlp_lib)

            # ---- constants
            idx16_sb = cst.tile([P, meta["W16"]], I16)
            nc.sync.dma_start(out=idx16_sb[:], in_=idx16_ap[:])
            x0loc_sb = cst.tile([P, T, F0], BF16)
            nc.sync.dma_start(out=x0loc_sb[:],
                              in_=x0loc_ap[:].rearrange("p (t f) -> p t f",
                                                        f=F0))
            dinv_sb = cst.tile([P, T], F32)
            nc.sync.dma_start(out=dinv_sb[:], in_=dinvT_ap[:])
            dinv2_sb = cst.tile([P, T], F32)
            nc.sync.dma_start(out=dinv2_sb[:], in_=dinv2T_ap[:])
            invd_sb = cst.tile([1, NP], BF16)
            nc.sync.dma_start(out=invd_sb[:], in_=invd_ap[:])
            brows_sb = cst.tile([1, 3, H], BF16)
            nc.sync.dma_start(out=brows_sb[:],
                              in_=brows_ap[:].rearrange("o (l h) -> o l h",
                                                        h=H))
            gb_sb = cst.tile([P, 6, H], F32)
            nc.sync.dma_start(out=gb_sb[:], in_=gb_ap[:])
            W1_sb = cst.tile([P, H], BF16)
            nc.sync.dma_start(out=W1_sb[:], in_=W1_ap[:])
            W2_sb = cst.tile([P, KH, H], BF16)
            nc.sync.dma_start(out=W2_sb[:],
                              in_=W2_ap[:].rearrange("(k p) h -> p k h", p=P))
            W3_sb = cst.tile([P, KH, H], BF16)
            nc.sync.dma_start(out=W3_sb[:],
                              in_=W3_ap[:].rearrange("(k p) h -> p k h", p=P))
            poolm_sb = cst.tile([P, T, G], BF16)
            nc.sync.dma_start(out=poolm_sb[:],
                              in_=poolm_ap[:].rearrange("p (t g) -> p t g",
                                                        g=G))
            fc1_sb = cst.tile([P, 3, H], F32)
            nc.sync.dma_start(out=fc1_sb[:],
                              in_=fc1_ap[:].rearrange("(k p) h -> p k h", p=P))
            attr_sb = cst.tile([P, G], F32)
            nc.sync.dma_start(out=attr_sb[:], in_=attr_ap[:])
            invc_sb = cst.tile([G, 1], F32)
            nc.sync.dma_start(out=invc_sb[:], in_=invc_ap[:])
            fcw2_sb = cst.tile([1, H], F32)
            nc.sync.dma_start(out=fcw2_sb[:], in_=fcw2_ap[:])
            fcb2_sb = cst.tile([G, 1], F32)
            nc.sync.dma_start(out=fcb2_sb[:], in_=fcb2_ap[:])
            ident_sb = cst.tile([P, P], F32)
            nc.sync.dma_start(out=ident_sb[:], in_=ident_ap[:])

            eps_sb = cst.tile([P, 1], F32)
            nc.vector.memset(eps_sb[:], 1e-5)
            zero_sb = cst.tile([P, 1], F32)
            nc.vector.memset(zero_sb[:], 0.0)
            ones_sb = cst.tile([1, G], F32)
            nc.vector.memset(ones_sb[:], 1.0)
            pool_acc = cst.tile([G, H], F32)
            nc.vector.memset(pool_acc[:], 0.0)
            ylocal = [cst.tile([P, T, H], F8, name=f"ylocal{i}")
                      for i in range(2)]

            # zero msg pools once (stale tail chunks x S zero-cols -> 0)
            for _ in range(LA + 3):
                m0 = msgA_p.tile([P, int(J[:, 0].max()), H], F8, tag="msgA")
                nc.vector.memset(m0[:], 0.0)
            for _ in range(4):
                m0 = msgB_p.tile([P, int(max(1, J[:, 1].max())), H], F8,
                                 tag="msgB")
                nc.vector.memset(m0[:], 0.0)

            regs = {}
            for v in sorted({int(J[t, h]) * P for t in range(T)
                             for h in range(2) if J[t, h] > 0}):
                regs[v] = nc.gpsimd.to_reg(v)

            # offsets into idx16 / S slabs
            o16 = np.zeros((T, 2), np.int64)
            och = np.zeros(T, np.int64)
            a16 = 0
            ach = 0
            for t in range(T):
                och[t] = ach
                for h in range(2):
                    o16[t, h] = a16
                    a16 += int(J[t, h]) * 8
                    ach += int(J[t, h])
                ach += 1  # self chunk

            Wsb = [W1_sb, W2_sb, W3_sb]
            Sap = [S0_ap, S12_ap, S12_ap]
            Sdt = [BF16, F8, F8]
            gq = [0]

            def emit_gA(L, t, msgs):
                Ja = int(J[t, 0])
                Fel = F0 if L == 0 else H
                dt = BF16 if L == 0 else F8
                m = msgA_p.tile([P, Ja, Fel], dt, tag="msgA")
                msgs[t] = m
                tab = (xa_ap[:] if L == 0 else h_full_a[L - 1][:])
                if "nogather" not in ablate:
                    nc.gpsimd.dma_gather(
                        out_ap=m[:], in_ap=tab,
                        idxs_ap=idx16_sb[:,
                                         int(o16[t, 0]):int(o16[t, 0]) + Ja * 8],
                        num_idxs=Ja * P, num_idxs_reg=regs[Ja * P],
                        elem_size=Fel, single_packet=False,
                        queue_num=gq[0] % nq)
                    gq[0] += 1

            def emit_gB(L, t, msgs):
                Jb = int(J[t, 1])
                Fel = F0 if L == 0 else H
                dt = BF16 if L == 0 else F8
                m = msgB_p.tile([P, max(Jb, 1), Fel], dt, tag="msgB")
                msgs[t] = m
                tab = (xb_ap[:] if L == 0 else h_full_b[L - 1][:])
                if Jb > 0 and "nogather" not in ablate:
                    nc.gpsimd.dma_gather(
                        out_ap=m[:, :Jb, :], in_ap=tab,
                        idxs_ap=idx16_sb[:,
                                         int(o16[t, 1]):int(o16[t, 1]) + Jb * 8],
                        num_idxs=Jb * P, num_idxs_reg=regs[Jb * P],
                        elem_size=Fel, single_packet=False,
                        queue_num=gq[0] % nq)
                    gq[0] += 1

            def emit_ag(L, which):
                if "noag" in ablate:
                    return
                if which == 0:
                    nc.gpsimd.collective_compute(
                        "AllGather", OP.bypass, replica_groups=rg,
                        ins=[h_cc_a[L][:]], outs=[h_full_a[L][:]])
                else:
                    nc.gpsimd.collective_compute(
                        "AllGather", OP.bypass, replica_groups=rg,
                        ins=[h_cc_b[L][:]], outs=[h_full_b[L][:]])

            for L in range(3):
                Fel = F0 if L == 0 else H
                KF = 1 if L == 0 else KH
                Sdtype = Sdt[L]
                yloc = x0loc_sb if L == 0 else ylocal[(L - 1) % 2]
                msgsA, msgsB = {}, {}
                for t in range(min(LA, T)):
                    emit_gA(L, t, msgsA)
                for t in range(T):
                    if t + LA < T:
                        emit_gA(L, t + LA, msgsA)
                    emit_gB(L, t, msgsB)
                    Ja, Jb = int(J[t, 0]), int(J[t, 1])
                    Jt = Ja + Jb
                    # S slab slice for this tile (A chunks, B chunks, self)
                    S_sb = sp.tile([P, Jt + 1, P], Sdtype, tag="S")
                    nc.sync.dma_start(
                        out=S_sb[:],
                        in_=Sap[L][:, int(och[t]) * P:
                                   (int(och[t]) + Jt + 1) * P].rearrange(
                            "p (j d) -> p j d", d=P))
                    mA, mB = msgsA.pop(t), msgsB.pop(t)

                    zt = psz.tile([P, KF, P], F32, tag="zt")
                    nagg = 0 if "noagg" in ablate else Jt + 1
                    for k in range(KF):
                        fs = slice(k * P, (k + 1) * P)
                        for j in range(nagg + (1 if nagg == 0 else 0)):
                            if j < Ja:
                                lhs = mA[:, j, fs]
                            elif j < Jt:
                                lhs = mB[:, j - Ja, fs]
                            else:
                                lhs = yloc[:, t, fs]
                            nc.tensor.matmul(zt[:, k, :], lhsT=lhs,
                                             rhs=S_sb[:, j, :],
                                             start=(j == 0),
                                             stop=(j == (nagg or 1) - 1))

                    zts = sbw.tile([P, KF, P], BF16, tag="zts")
                    nc.scalar.activation(out=zts[:], in_=zt[:], func=AF.Copy)

                    psy = psyp.tile([P, H], F32, tag="psy")
                    for k in range(KF):
                        nc.tensor.matmul(psy[:], lhsT=zts[:, k, :],
                                         rhs=(Wsb[L][:, :] if L == 0
                                              else Wsb[L][:, k, :]),
                                         start=(k == 0), stop=False)
                    nc.tensor.matmul(psy[:],
                                     lhsT=invd_sb[:, t * P:(t + 1) * P],
                                     rhs=brows_sb[:, L, :],
                                     start=False, stop=True)

                    # ---- LayerNorm on conv = dinv * psy, fused ReLU
                    stats = small.tile([P, 6], F32, tag="stats")
                    nc.vector.bn_stats(out=stats[:], in_=psy[:])
                    mv = small.tile([P, 2], F32, tag="mv")
                    nc.vector.bn_aggr(out=mv[:], in_=stats[:])
                    vc = small.tile([P, 1], F32, tag="vc")
                    nc.vector.tensor_tensor(out=vc[:], in0=mv[:, 1:2],
                                            in1=dinv2_sb[:, t:t + 1],
                                            op=OP.mult)
                    sd = small.tile([P, 1], F32, tag="sd")
                    nc.scalar.activation(out=sd[:], in_=vc[:], func=AF.Sqrt,
                                         bias=eps_sb[:], scale=1.0)
                    rc = small.tile([P, 1], F32, tag="rc")
                    nc.vector.reciprocal(out=rc[:], in_=sd[:])
                    sc = small.tile([P, 1], F32, tag="sc")
                    nc.vector.tensor_tensor(out=sc[:], in0=rc[:],
                                            in1=dinv_sb[:, t:t + 1],
                                            op=OP.mult)
                    nm = small.tile([P, 1], F32, tag="nm")
                    nc.vector.tensor_tensor(out=nm[:], in0=mv[:, 0:1],
                                            in1=sc[:], op=OP.mult)
                    nb = small.tile([P, 1], F32, tag="nb")
                    nc.vector.tensor_tensor(out=nb[:], in0=zero_sb[:],
                                            in1=nm[:], op=OP.subtract)

                    if not (gamma_trivial and beta_trivial):
                        # generic path: normalize (no relu), apply g/b, relu
                        tt = sbw.tile([P, H], F32, tag="tt")
                        nc.scalar.activation(out=tt[:], in_=psy[:],
                                             func=AF.Copy, bias=nb[:],
                                             scale=sc[:])
                        if not gamma_trivial:
                            nc.vector.tensor_mul(out=tt[:], in0=tt[:],
                                                 in1=gb_sb[:, 2 * L, :])
                        if not beta_trivial:
                            nc.vector.tensor_add(out=tt[:], in0=tt[:],
                                                 in1=gb_sb[:, 2 * L + 1, :])
                        src_ap, src_sc, src_nb = tt, None, None
                    else:
                        src_ap = None

                    if L < 2:
                        sc8 = small.tile([P, 1], F32, tag="sc8")
                        nc.vector.tensor_tensor(out=sc8[:], in0=sc[:],
                                                in1=dinv_sb[:, t:t + 1],
                                                op=OP.mult)
                        nb8 = small.tile([P, 1], F32, tag="nb8")
                        nc.vector.tensor_tensor(out=nb8[:], in0=nb[:],
                                                in1=dinv_sb[:, t:t + 1],
                                                op=OP.mult)
                        ydst = ylocal[L % 2][:, t, :]
                        if src_ap is None:
                            nc.scalar.activation(out=ydst, in_=psy[:],
                                                 func=AF.Relu, bias=nb8[:],
                                                 scale=sc8[:])
                        else:
                            nc.scalar.activation(out=ydst, in_=src_ap[:],
                                                 func=AF.Relu, bias=0.0,
                                                 scale=dinv_sb[:, t:t + 1])
                        if t < TA:
                            nc.sync.dma_start(
                                out=h_cc_a[L][t * P:(t + 1) * P, :], in_=ydst)
                        else:
                            tb = t - TA
                            nc.sync.dma_start(
                                out=h_cc_b[L][tb * P:(tb + 1) * P, :],
                                in_=ydst)
                        if t == TA - 1:
                            emit_ag(L, 0)
                        elif t == T - 1:
                            emit_ag(L, 1)
                    else:
                        h2 = sbw.tile([P, H], BF16, tag="h2")
                        if src_ap is None:
                            nc.scalar.activation(out=h2[:], in_=psy[:],
                                                 func=AF.Relu, bias=nb[:],
                                                 scale=sc[:])
                        else:
                            nc.scalar.activation(out=h2[:], in_=src_ap[:],
                                                 func=AF.Relu)
                        pp = ptail.tile([G, H], F32, tag="tail")
                        nc.tensor.matmul(pp[:], lhsT=poolm_sb[:, t, :],
                                         rhs=h2[:], start=True, stop=True)
                        nc.vector.tensor_add(out=pool_acc[:],
                                             in0=pool_acc[:], in1=pp[:])

            # ---------- pooled mean + MLP head ----------
            nc.sync.dma_start(out=pool_in[:], in_=pool_acc[:])
            nc.gpsimd.collective_compute(
                "AllReduce", OP.add, replica_groups=rg,
                ins=[pool_in[:]], outs=[pool_out[:]])
            pooled = sbw.tile([G, H], F32, tag="pooled")
            nc.sync.dma_start(out=pooled[:], in_=pool_out[:])
            nc.vector.tensor_scalar_mul(out=pooled[:], in0=pooled[:],
                                        scalar1=invc_sb[:])
            zt2 = sbw.tile([P, KH, G], F32, tag="zt2")
            for kk in range(KH):
                pz = ptail.tile([P, G], F32, tag="tail")
                nc.tensor.transpose(out=pz[:],
                                    in_=pooled[:, kk * P:(kk + 1) * P],
                                    identity=ident_sb[:G, :G])
                nc.vector.tensor_copy(out=zt2[:, kk, :], in_=pz[:])
            ups = ptail.tile([G, H], F32, tag="tail")
            nc.tensor.matmul(ups[:], lhsT=zt2[:, 0, :], rhs=fc1_sb[:, 0, :],
                             start=True, stop=False)
            nc.tensor.matmul(ups[:], lhsT=zt2[:, 1, :], rhs=fc1_sb[:, 1, :],
                             start=False, stop=False)
            nc.tensor.matmul(ups[:], lhsT=attr_sb[:], rhs=fc1_sb[:, 2, :],
                             start=False, stop=True)
            r = sbw.tile([G, H], F32, tag="r")
            nc.scalar.activation(out=r[:], in_=ups[:], func=AF.Relu)
            wps = ptail.tile([G, H], F32, tag="tail")
            nc.tensor.matmul(wps[:], lhsT=ones_sb[:], rhs=fcw2_sb[:],
                             start=True, stop=True)
            rr = sbw.tile([G, H], F32, tag="rr")
            nc.vector.tensor_mul(out=rr[:], in0=r[:], in1=wps[:])
            o = small.tile([G, 1], F32, tag="o")
            nc.vector.tensor_reduce(out=o[:], in_=rr[:],
                                    axis=mybir.AxisListType.X, op=OP.add)
            nc.vector.tensor_scalar_add(out=o[:], in0=o[:],
                                        scalar1=fcb2_sb[:])
            nc.sync.dma_start(out=out_ap[:], in_=o[:])

    nc.compile()
    return nc


def make_in_maps(cfg, inputs, per_core, meta):
    import ml_dtypes
    H, G, A = cfg.H, cfg.G, cfg.A
    f = lambda a: np.ascontiguousarray(np.asarray(a, np.float32))
    W1, b1 = f(inputs["W1"]), f(inputs["b1"])
    W2, b2 = f(inputs["W2"]), f(inputs["b2"])
    W3, b3 = f(inputs["W3"]), f(inputs["b3"])
    g1, be1 = f(inputs["g1"]), f(inputs["be1"])
    g2, be2 = f(inputs["g2"]), f(inputs["be2"])
    g3, be3 = f(inputs["g3"]), f(inputs["be3"])
    fcW1, fcb1 = f(inputs["fcW1"]), f(inputs["fcb1"])
    fcW2, fcb2 = f(inputs["fcW2"]), f(inputs["fcb2"])
    graph_attr = f(inputs["graph_attr"]).reshape(-1, A)
    batch = np.asarray(inputs["batch"], np.int64)

    brows = np.zeros((1, 3 * H), np.float32)
    brows[0, :H] = b1
    brows[0, H:2 * H] = b2
    brows[0, 2 * H:] = b3
    gb = np.zeros((P, 6, H), np.float32)
    for i, v in enumerate([g1, be1, g2, be2, g3, be3]):
        gb[:, i, :] = v[None, :]
    fc1aug = np.zeros((3 * P, H), np.float32)
    fc1aug[:H, :] = fcW1[:H, :]
    fc1aug[2 * P:2 * P + A, :] = fcW1[H:H + A, :]
    fc1aug[2 * P + A, :] = fcb1
    attraug = np.zeros((P, G), np.float32)
    attraug[:A, :] = graph_attr.T
    attraug[A, :] = 1.0
    cnt = np.bincount(batch, minlength=G).astype(np.float32)
    invc = (1.0 / np.maximum(cnt, 1.0)).reshape(G, 1).astype(np.float32)
    fcw2row = fcW2[:, 0].reshape(1, H).copy()
    fcb2col = np.full((G, 1), fcb2[0], np.float32)
    ident_in = np.eye(P, dtype=np.float32)

    shared = dict(brows=brows.astype(ml_dtypes.bfloat16), gb=gb,
                  W1=W1.astype(ml_dtypes.bfloat16),
                  W2=W2.astype(ml_dtypes.bfloat16),
                  W3=W3.astype(ml_dtypes.bfloat16),
                  fc1aug=fc1aug, attraug=attraug, invc=invc,
                  fcw2row=fcw2row, fcb2col=fcb2col, ident_in=ident_in)
    in_maps = []
    for c in range(cfg.M):
        m = dict(shared)
        m.update(per_core[c])
        in_maps.append(m)
    return in_maps


_CACHE = {}


def _get_program(cfg, meta, gamma_trivial, beta_trivial):
    key = (tuple(meta["J"].reshape(-1).tolist()), gamma_trivial, beta_trivial)
    if key not in _CACHE:
        _CACHE[key] = build_program(cfg, meta, gamma_trivial, beta_trivial)
    return _CACHE[key]


def run(cfg, inputs, nc=None):
    per_core, meta = preprocess(cfg, inputs["x"], inputs["edge_index"],
                                inputs["batch"])
    gamma_trivial = all(np.allclose(np.asarray(inputs[k]), 1.0)
                        for k in ("g1", "g2", "g3"))
    beta_trivial = all(np.allclose(np.asarray(inputs[k]), 0.0)
                       for k in ("be1", "be2", "be3"))
    if nc is None:
        nc = _get_program(cfg, meta, gamma_trivial, beta_trivial)
    in_maps = make_in_maps(cfg, inputs, per_core, meta)
    res = None
    for attempt in range(3):
        try:
            res = run_bass_kernel_spmd(nc, in_maps, list(range(cfg.M)))
            break
        except Exception:
            if attempt == 2:
                raise
    return res.results[0]["out"].astype(np.float32)


def kernel(**inputs) -> np.ndarray:
    return run(CFG, inputs)
